# revision 4
# baseline (speedup 1.0000x reference)
"""Trainium2 Bass kernel for nn_CausalAggregator.

Computes, for target stocks y:
    out[y, :] = Beta[:, y] @ concat([X, adjacency[:, y, :]], 1) @ W + bias
              = (Beta.T @ X) @ Wf  +  (einsum('ny,nyc->yc', Beta, adj)) @ Wa + bias

Sharding: split Beta / adjacency along the target axis y across 8 cores;
replicate X, weight, bias. Each core computes 512 output rows; no
cross-device reduction.

The baseline fp32 version was DMA-bound (36.5 MB/core @ 358 GB/s ≈ 100 us)
and DVE-bound (fp32 tensor ops run in 1x mode). The output tolerance is
2e-2, so inputs are quantized on the host: adjacency/Beta to fp8-E3M4
(both are uniform [0,1] — E3M4's 4 mantissa bits give ~1.5% max elem error
and the terms average over 4096 sources), X/weight to bf16. fp8 tensors
are upcast to bf16 during the HBM->SBUF DMA (SWDGE cast), which keeps the
DVE in its 2x bf16 mode. Adjacency is shipped pre-transposed as [N, C, Y]
so the DVE product and the PE ones-reduce are unit-stride.

Per-core (N=4096 n-tiles of 128, Y=512, D=O=256, C=3):
  stream 32 n-tiles:
    GT_psum[d_t] += X_tile[:, d_t].T @ Beta_tile      (PE, K-accum in PSUM)
    prod = adj_tile * Beta_tile (bcast over c)        (DVE mul, bf16 2x)
    red_psum[c]  += ones.T @ prod[:, c-block]         (PE ones-reduce)
  epilogue:
    out[y_t] = GT.T @ Wf + adjaggT.T @ Wa + bias      (PSUM accum)
"""

import numpy as np

import concourse.mybir as mybir
import concourse.tile as tile
from concourse import bacc
from concourse.bass import ds, ts
from concourse.bass_utils import run_bass_kernel_spmd

P = 128
F32 = mybir.dt.float32
BF16 = mybir.dt.bfloat16
FP8 = mybir.dt.float8e3

# Full problem shapes (hardcoded; kernel.py must be self-contained).
N_FULL = 4096   # source stocks (contraction axis)
Y_TOTAL = 4096  # target stocks (sharded)
D_FULL = 256    # input features
O_FULL = 256    # output features
C_FULL = 3      # adjacency channels
N_CORES = 8
Y_FULL = Y_TOTAL // N_CORES  # per-core target slice

# HBM dtypes (SBUF compute is always bf16)
ADJ_DT = BF16
BETA_DT = BF16
X_DT = BF16


def emit_causal_agg(tc, io, N, Y, D, O, C, io_bufs=4):
    nc = tc.nc
    beta, adj, x, w, bias, out = (
        io["beta"], io["adj"], io["x"], io["w"], io["bias"], io["out"])

    n_nt, n_yt, n_dt = N // P, Y // P, D // P
    CY = C * Y
    assert Y <= 1024

    adj_flat = adj.rearrange("n c y -> n (c y)")

    def dma_in(eng, out_t, in_ap):
        # SWDGE (gpsimd) is required when the HBM dtype differs from SBUF
        if out_t.dtype != in_ap.dtype:
            nc.gpsimd.dma_start(out=out_t, in_=in_ap)
        else:
            eng.dma_start(out=out_t, in_=in_ap)

    with (
        tc.tile_pool(name="const", bufs=1) as cpool,
        tc.tile_pool(name="io", bufs=io_bufs) as iopool,
        tc.tile_pool(name="prod", bufs=3) as ppool,
        tc.tile_pool(name="fin", bufs=1) as fpool,
        tc.tile_pool(name="osb", bufs=2) as opool,
    ):
        # --- constants ---
        ones = cpool.tile([P, 1], BF16, tag="ones")
        nc.vector.memset(ones, 1.0)
        wf = []
        for d_t in range(n_dt):
            t = cpool.tile([P, O], BF16, tag=f"wf{d_t}", name=f"wf{d_t}")
            nc.sync.dma_start(out=t, in_=w[ts(d_t, P), :])
            wf.append(t)
        wa = cpool.tile([C, O], BF16, tag="wa")
        nc.sync.dma_start(out=wa, in_=w[D:D + C, :])
        bias_bc = cpool.tile([P, O], F32, tag="bias")
        nc.sync.dma_start(out=bias_bc, in_=bias.unsqueeze(0).to_broadcast((P, O)))

        gt_sb = [fpool.tile([P, Y], BF16, tag=f"gt{d_t}", name=f"gt{d_t}")
                 for d_t in range(n_dt)]
        red_sb = fpool.tile([1, CY], BF16, tag="redsb", name="redsb")
        adjaggT_sb = fpool.tile([C, Y], BF16, tag="adjaggT", name="adjaggT")

        # --- main streaming loop: PSUM accumulation over n-tiles ---
        with tc.tile_pool(name="acc", bufs=1, space="PSUM") as accpool:
            gt_psum = [accpool.tile([P, Y], F32, tag=f"gtp{d_t}", name=f"gtp{d_t}")
                       for d_t in range(n_dt)]
            red_psum = [accpool.tile([1, Y], F32, tag=f"red{c}", name=f"red{c}")
                        for c in range(C)]

            for n_t in range(n_nt):
                first, last = n_t == 0, n_t == n_nt - 1
                beta_t = iopool.tile([P, Y], BF16, tag="beta")
                dma_in(nc.scalar, beta_t, beta[ts(n_t, P), :])
                x_t = iopool.tile([P, D], BF16, tag="x")
                dma_in(nc.scalar, x_t, x[ts(n_t, P), :])
                adj_t = iopool.tile([P, CY], BF16, tag="adj")
                dma_in(nc.sync, adj_t, adj_flat[ts(n_t, P), :])

                for d_t in range(n_dt):
                    nc.tensor.matmul(gt_psum[d_t], x_t[:, ts(d_t, P)], beta_t,
                                     start=first, stop=last)

                prod_t = ppool.tile([P, CY], BF16, tag="prod")
                beta_bc = beta_t.unsqueeze(1).to_broadcast((P, C, Y))
                nc.vector.tensor_mul(
                    prod_t.rearrange("p (c y) -> p c y", c=C),
                    adj_t.rearrange("p (c y) -> p c y", c=C),
                    beta_bc)
                for c in range(C):
                    nc.tensor.matmul(red_psum[c], ones, prod_t[:, ts(c, Y)],
                                     start=first, stop=last)

            # drain accumulators to SBUF
            for d_t in range(n_dt):
                nc.any.tensor_copy(gt_sb[d_t], gt_psum[d_t])
            for c in range(C):
                nc.any.tensor_copy(red_sb[:, ts(c, Y)], red_psum[c])

        # compute engines can't write rows at partition offset 1/2; DMA can
        for c in range(C):
            nc.sync.dma_start(out=adjaggT_sb[c:c + 1, :], in_=red_sb[:, ts(c, Y)])

        # --- epilogue: out[y_t] = GT.T @ Wf + adjaggT.T @ Wa + bias ---
        with tc.tile_pool(name="fpsum", bufs=2, space="PSUM") as fpsum_pool:
            for y_t in range(n_yt):
                f_psum = fpsum_pool.tile([P, O], F32, tag="fpsum")
                for d_t in range(n_dt):
                    nc.tensor.matmul(f_psum, gt_sb[d_t][:, ts(y_t, P)], wf[d_t],
                                     start=(d_t == 0), stop=False)
                nc.tensor.matmul(f_psum, adjaggT_sb[:, ts(y_t, P)], wa,
                                 start=False, stop=True)
                o_sb = opool.tile([P, O], F32, tag="osb")
                nc.vector.tensor_add(o_sb, f_psum, bias_bc)
                nc.sync.dma_start(out=out[ts(y_t, P), :], in_=o_sb)


def build_nc(N=N_FULL, Y=Y_FULL, D=D_FULL, O=O_FULL, C=C_FULL, reps=1,
             internal_inputs=False, **flags):
    nc = bacc.Bacc("TRN2", target_bir_lowering=False, debug=False)
    kind = "Internal" if internal_inputs else "ExternalInput"
    io = {
        "beta": nc.dram_tensor("beta", [N, Y], BETA_DT, kind=kind).ap(),
        "adj": nc.dram_tensor("adj", [N, C, Y], ADJ_DT, kind=kind).ap(),
        "x": nc.dram_tensor("x", [N, D], X_DT, kind=kind).ap(),
        "w": nc.dram_tensor("w", [D + C, O], BF16, kind=kind).ap(),
        "bias": nc.dram_tensor("bias", [O], F32, kind=kind).ap(),
        "out": nc.dram_tensor("out", [Y, O], F32, kind="ExternalOutput").ap(),
    }
    with tile.TileContext(nc) as tc:
        for _ in range(reps):
            emit_causal_agg(tc, io, N, Y, D, O, C, **flags)
    nc.compile()
    return nc


def make_in_maps(adjacency, input_feature, Beta, weight, bias):
    """Host-side shard + quantize. Returns per-core input dicts."""
    np_adj = mybir.dt.np(ADJ_DT)
    np_beta = mybir.dt.np(BETA_DT)
    np_x = mybir.dt.np(X_DT)
    np_bf = mybir.dt.np(BF16)

    x16 = np.ascontiguousarray(np.asarray(input_feature)).astype(np_x)
    w16 = np.ascontiguousarray(np.asarray(weight)).astype(np_bf)
    bias32 = np.ascontiguousarray(np.asarray(bias, dtype=np.float32))
    adjacency = np.asarray(adjacency)
    Beta = np.asarray(Beta)

    in_maps = []
    for i in range(N_CORES):
        ys = slice(i * Y_FULL, (i + 1) * Y_FULL)
        # adjacency slice transposed to [N, C, Y] for unit-stride DVE ops
        adj_i = adjacency[:, ys, :].transpose(0, 2, 1).astype(np_adj)
        beta_i = Beta[:, ys].astype(np_beta)
        in_maps.append({
            "beta": np.ascontiguousarray(beta_i),
            "adj": np.ascontiguousarray(adj_i),
            "x": x16,
            "w": w16,
            "bias": bias32,
        })
    return in_maps


_NC_CACHE = None


def _get_nc():
    global _NC_CACHE
    if _NC_CACHE is None:
        _NC_CACHE = build_nc()
    return _NC_CACHE


def run(adjacency, input_feature, Beta, weight, bias, trace=False):
    nc = _get_nc()
    in_maps = make_in_maps(adjacency, input_feature, Beta, weight, bias)
    res = run_bass_kernel_spmd(nc, in_maps, core_ids=list(range(N_CORES)),
                               trace=trace)
    out = np.concatenate([res.results[i]["out"] for i in range(N_CORES)], axis=0)
    return out, res


def kernel(adjacency, input_feature, Beta, weight, bias):
    out, _ = run(adjacency, input_feature, Beta, weight, bias, trace=False)
    return out
